# revision 7
# baseline (speedup 1.0000x reference)
"""Bidirectional GRU encoder (Keras reset_after GRU, mask_zero) on 8 trn2 cores.

Sharding: 2 directions x 4 batch-quarters (16 examples/core). Everything is
computed feature-major ("transposed"): hidden state, gates and outputs live as
[feature partitions, batch free], so the recurrent matmul
    gr^T[m,b] = sum_k U[k,m] * h[k,b]
uses U tiles as the stationary operand (bf16, FWL weight loads) and the
per-example hidden state as the tiny moving operand, and no transposes are
ever needed inside the scan.

Per core:
  - embed: dma_gather(transpose=True) from a bf16 copy of the table in SBUF
  - gx = xe @ W precomputed per 32-step chunk, interleaved with the scan
  - mask (token==0 -> carry state) folded into the z-gate pre-activation via a
    K=1 matmul adding +30 (sigmoid(30)~=1 => h_t = h_{t-1})
  - scan: 256 steps, 24 gate-tiles x 8 k-tiles of bf16 matmuls into PSUM,
    sigmoid/tanh on ScalarE, update chain on VectorE
  - final state partial: h_last @ Wp_half in fp32

Host: builds indices/masks, slices batch, assembles outputs, and computes
state = tanh(s_fwd + s_bwd + bp) from the two per-direction partial products
(elementwise only; both matmul halves are computed on device).
"""
import sys

for _p in ("/opt/trn_rl_repo",):
    if _p not in sys.path:
        sys.path.insert(0, _p)

import numpy as np
import ml_dtypes

import concourse.bass as bass
import concourse.tile as tile
from concourse import bacc, mybir

BF = mybir.dt.bfloat16
F32 = mybir.dt.float32
AF = mybir.ActivationFunctionType

VOCAB, EMB, H, B, T = 32000, 256, 1024, 64, 256
NCORES = 8
BL = B // 4            # batch per core (2 dirs x 4 quarters)
TOK = BL * T           # tokens gathered per core
TC = 32                # scan steps per chunk
NCHUNK = T // TC
KT = H // 128          # 8 k-tiles
MT = 3 * H // 128      # 24 gate m-tiles
GXTOK = 512            # tokens per gx chunk = TC * BL
MASK_BIG = 30.0


def build_program(t_steps=T, with_bias_zr=False, with_bias_rh=False):
    assert t_steps % TC == 0
    nchunk = t_steps // TC
    tok = BL * t_steps
    nc = bacc.Bacc("TRN2", target_bir_lowering=False, debug=False,
                   num_devices=NCORES)
    d = {}
    d["u"] = nc.dram_tensor("u", [H, 3 * H], F32, kind="ExternalInput").ap()
    d["w"] = nc.dram_tensor("w", [EMB, 3 * H], F32, kind="ExternalInput").ap()
    d["emb"] = nc.dram_tensor("emb", [VOCAB, EMB], F32, kind="ExternalInput").ap()
    d["idx"] = nc.dram_tensor("idx", [128, tok // 16], mybir.dt.int16,
                              kind="ExternalInput").ap()
    d["mrow"] = nc.dram_tensor("mrow", [1, tok], F32, kind="ExternalInput").ap()
    d["h0T"] = nc.dram_tensor("h0T", [H, BL], F32, kind="ExternalInput").ap()
    d["wp"] = nc.dram_tensor("wp", [H, H], F32, kind="ExternalInput").ap()
    if with_bias_zr:
        d["bzr"] = nc.dram_tensor("bzr", [1, 3 * H], F32, kind="ExternalInput").ap()
    if with_bias_rh:
        d["brh"] = nc.dram_tensor("brh", [H, 1], F32, kind="ExternalInput").ap()
    outT = nc.dram_tensor("outT", [nchunk, KT, 128, TC, BL], BF,
                          kind="ExternalOutput").ap()
    sp = nc.dram_tensor("sp", [128, 8 * BL], F32, kind="ExternalOutput").ap()

    with tile.TileContext(nc) as tc:
        with tc.tile_pool(name="persist", bufs=1) as pp, \
             tc.tile_pool(name="gxp", bufs=2) as gxp, \
             tc.tile_pool(name="outp", bufs=2) as outp, \
             tc.tile_pool(name="step", bufs=3) as stp, \
             tc.tile_pool(name="psg", bufs=2, space="PSUM") as psg, \
             tc.tile_pool(name="psx", bufs=2, space="PSUM") as psx:

            # ---------- setup: indices, mask row, gather table ----------
            idxs = pp.tile([128, tok // 16], mybir.dt.int16)
            nc.sync.dma_start(idxs[:], d["idx"][:])
            mrow = pp.tile([1, tok], BF)
            nc.gpsimd.dma_start(mrow[:], d["mrow"][:])
            ones30 = pp.tile([1, 128], BF)
            nc.vector.memset(ones30[:], 1.0)

            xeT = pp.tile([128, 2, tok], BF)
            emb_bf = nc.dram_tensor("emb_bf", [VOCAB, EMB], BF, kind="Internal").ap()
            emb_r = d["emb"].rearrange("(s p) e -> p s e", p=128)
            emb_bf_r = emb_bf.rearrange("(s p) e -> p s e", p=128)
            NS = VOCAB // 128           # 250 stripes
            with tc.tile_pool(name="embp", bufs=2) as embp:
                for s0 in range(0, NS, 10):
                    stage = embp.tile([128, 10, EMB], BF, tag="estage")
                    nc.gpsimd.dma_start(stage[:], emb_r[:, s0:s0 + 10, :])
                    nc.sync.dma_start(emb_bf_r[:, s0:s0 + 10, :], stage[:])
            nc.gpsimd.dma_gather(
                xeT[:], emb_bf[:], idxs[:],
                num_idxs=tok, num_idxs_reg=tok, elem_size=EMB, transpose=True)

            # ---------- weights (loaded into space freed by emb) ----------
            U = pp.tile([128, KT, 3 * H], BF)
            nc.gpsimd.dma_start(U[:], d["u"].rearrange("(k p) n -> p k n", p=128))
            W = pp.tile([128, 2, 3 * H], BF)
            nc.gpsimd.dma_start(W[:], d["w"].rearrange("(k p) n -> p k n", p=128))
            Wp = pp.tile([128, KT, H], F32)
            nc.sync.dma_start(Wp[:], d["wp"].rearrange("(k p) n -> p k n", p=128))
            h0 = pp.tile([128, KT, BL], BF)
            nc.gpsimd.dma_start(h0[:], d["h0T"].rearrange("(k p) b -> p k b", p=128))
            if with_bias_zr:
                bzr = pp.tile([1, 3 * H], BF)
                nc.gpsimd.dma_start(bzr[:], d["bzr"][:])
            if with_bias_rh:
                brh = pp.tile([128, 8], F32)
                nc.gpsimd.dma_start(
                    brh[:], d["brh"].rearrange("(m p) o -> p (m o)", p=128))

            # ---------- gx chunk computation (emitted inline) ----------
            def emit_gx_group(c, m, gx_sb):
                """gate pre-activations from the input side, chunk c, m-tile m."""
                ps = psx.tile([128, GXTOK], F32, tag="gx")
                ts = slice(c * GXTOK, (c + 1) * GXTOK)
                mms = [(W[:, 0, 128 * m:128 * (m + 1)], xeT[:, 0, ts]),
                       (W[:, 1, 128 * m:128 * (m + 1)], xeT[:, 1, ts])]
                if with_bias_zr:
                    mms.append((bzr[:, 128 * m:128 * (m + 1)], ones_row[:, ts]))
                if m < 8:  # z-gate tiles: add MASK_BIG * (token==0)
                    mms.append((ones30[:], mrow[:, ts]))
                for i, (lhsT, rhs) in enumerate(mms):
                    nc.tensor.matmul(ps[:], lhsT, rhs, start=(i == 0),
                                     stop=(i == len(mms) - 1))
                # -> gx_sb [128, TC, MT, BL] slot for this m
                nc.vector.tensor_copy(
                    gx_sb[:, :, m, :],
                    ps[:].rearrange("p (t b) -> p t b", t=TC))

            if with_bias_zr:
                ones_row = pp.tile([1, tok], BF)
                nc.vector.memset(ones_row[:], 1.0)

            gx_tiles = []

            def gx_chunk_tile():
                t_ = gxp.tile([128, TC, MT, BL], BF, tag="gx_sb")
                gx_tiles.append(t_)
                return t_

            # precompute chunk 0
            gx_cur = gx_chunk_tile()
            for m in range(MT):
                emit_gx_group(0, m, gx_cur)

            # ---------- the scan ----------
            hprev = h0[:, :, :]            # [128, KT, BL] view
            GB = 8 * BL                    # free size of one gate region
            out_cur = outp.tile([128, KT, TC, BL], BF, tag="outbuf")
            gx_next = None

            for t in range(t_steps):
                c, tt = divmod(t, TC)
                psum = psg.tile([128, MT * BL], F32, tag="gates")
                hp = hprev

                def scan_mm(m):
                    for k in range(KT):
                        nc.tensor.matmul(
                            psum[:, BL * m:BL * (m + 1)],
                            U[:, k, 128 * m:128 * (m + 1)],
                            hp[:, k, :],
                            start=(k == 0), stop=(k == KT - 1))

                for m in range(16):
                    scan_mm(m)
                # z,r pre-activations + sigmoid while PE does the h-bar tiles
                gzr = stp.tile([128, 2 * GB], BF, tag="gzr")
                nc.vector.tensor_add(gzr[:], psum[:, :2 * GB],
                                     gx_cur[:, tt, 0:16, :]
                                     .rearrange("p m b -> p (m b)"))
                zr = stp.tile([128, 2 * GB], BF, tag="zr")
                nc.scalar.activation(zr[:], gzr[:], AF.Sigmoid)

                for m in range(16, MT):
                    scan_mm(m)

                # interleave next chunk's gx into this chunk's steps
                if tt < MT and c + 1 < nchunk:
                    if tt == 0:
                        gx_next = gx_chunk_tile()
                    emit_gx_group(c + 1, tt, gx_next)

                rh = psum[:, 2 * GB:3 * GB]
                if with_bias_rh:
                    rhb = stp.tile([128, GB], F32, tag="rhb")
                    for j in range(8):
                        nc.vector.tensor_scalar_add(
                            rhb[:, BL * j:BL * (j + 1)],
                            rh[:, BL * j:BL * (j + 1)], brh[:, j:j + 1])
                    rh = rhb[:]
                t1 = stp.tile([128, GB], BF, tag="t1")
                nc.vector.tensor_mul(t1[:], zr[:, GB:], rh)
                t2 = stp.tile([128, GB], BF, tag="t2")
                nc.vector.tensor_add(t2[:], t1[:],
                                     gx_cur[:, tt, 16:24, :]
                                     .rearrange("p m b -> p (m b)"))
                hh = stp.tile([128, GB], BF, tag="hh")
                nc.scalar.activation(hh[:], t2[:], AF.Tanh)
                dd = stp.tile([128, GB], BF, tag="dd")
                nc.vector.tensor_sub(dd[:].rearrange("p (k b) -> p k b", k=KT),
                                     hp[:, :, :], 
                                     hh[:].rearrange("p (k b) -> p k b", k=KT))
                pz = stp.tile([128, GB], BF, tag="pz")
                nc.vector.tensor_mul(pz[:], zr[:, :GB], dd[:])
                nc.vector.tensor_add(out_cur[:, :, tt, :],
                                     hh[:].rearrange("p (k b) -> p k b", k=KT),
                                     pz[:].rearrange("p (k b) -> p k b", k=KT))
                hprev = out_cur[:, :, tt, :]

                if tt == TC - 1:
                    nc.sync.dma_start(
                        outT[c].rearrange("k p t b -> p k t b"),
                        out_cur[:])
                    if c + 1 < nchunk:
                        gx_cur = gx_next
                        out_prev, out_cur = out_cur, outp.tile(
                            [128, KT, TC, BL], BF, tag="outbuf")

            # ---------- final state partial: s^T = Wp_half^T @ h_last ----------
            hl = stp.tile([128, KT, BL], F32, tag="hl")
            nc.vector.tensor_copy(hl[:], hprev[:, :, :])
            pss = psx.tile([128, 8 * BL], F32, tag="st")
            for m in range(8):
                for k in range(KT):
                    nc.tensor.matmul(
                        pss[:, BL * m:BL * (m + 1)],
                        Wp[:, k, 128 * m:128 * (m + 1)],
                        hl[:, k, :],
                        start=(k == 0), stop=(k == KT - 1))
            ssb = stp.tile([128, 8 * BL], F32, tag="ssb")
            nc.vector.tensor_copy(ssb[:], pss[:])
            nc.sync.dma_start(sp[:], ssb[:])

    nc.compile()
    return nc


# ----------------------------------------------------------------------------
# host side
# ----------------------------------------------------------------------------
_CACHE = {}


def _get_program(t_steps, with_bias_zr, with_bias_rh):
    key = (t_steps, with_bias_zr, with_bias_rh)
    if key not in _CACHE:
        _CACHE[key] = build_program(t_steps, with_bias_zr, with_bias_rh)
    return _CACHE[key]


def _make_runner(nc, n_cores):
    import jax
    from jax.sharding import Mesh, PartitionSpec
    from jax.experimental.shard_map import shard_map
    from concourse.bass2jax import (_bass_exec_p, partition_id_tensor,
                                    install_neuronx_cc_hook)
    install_neuronx_cc_hook()
    partition_name = (nc.partition_id_tensor.name
                      if nc.partition_id_tensor else None)
    in_names, out_names, out_avals, zero_shapes = [], [], [], []
    for alloc in nc.m.functions[0].allocations:
        if not isinstance(alloc, mybir.MemoryLocationSet):
            continue
        name = alloc.memorylocations[0].name
        if alloc.kind == "ExternalInput":
            if name != partition_name:
                in_names.append(name)
        elif alloc.kind == "ExternalOutput":
            shape = tuple(alloc.tensor_shape)
            dtype = mybir.dt.np(alloc.dtype)
            out_names.append(name)
            out_avals.append(jax.core.ShapedArray(shape, dtype))
            zero_shapes.append((shape, dtype))
    n_params, n_outs = len(in_names), len(out_names)
    all_in_names = list(in_names) + list(out_names)
    if partition_name is not None:
        all_in_names.append(partition_name)
    donate = tuple(range(n_params, n_params + n_outs))

    def _body(*args):
        operands = list(args)
        if partition_name is not None:
            operands.append(partition_id_tensor())
        return tuple(_bass_exec_p.bind(
            *operands, out_avals=tuple(out_avals), in_names=tuple(all_in_names),
            out_names=tuple(out_names), lowering_input_output_aliases=(),
            sim_require_finite=True, sim_require_nnan=True, nc=nc))

    devices = jax.devices()[:n_cores]
    mesh = Mesh(np.asarray(devices), ("core",))
    fn = jax.jit(
        shard_map(_body, mesh=mesh,
                  in_specs=(PartitionSpec("core"),) * (n_params + n_outs),
                  out_specs=(PartitionSpec("core"),) * n_outs,
                  check_rep=False),
        donate_argnums=donate, keep_unused=True)

    def run(in_maps):
        concat_in = [np.concatenate([np.asarray(in_maps[c][n])
                                     for c in range(n_cores)], axis=0)
                     for n in in_names]
        zeros = [np.zeros((n_cores * s[0], *s[1:]), dt)
                 for (s, dt) in zero_shapes]
        out_arrs = fn(*concat_in, *zeros)
        import jax as _jax
        _jax.block_until_ready(out_arrs)
        return [{name: np.asarray(out_arrs[i]).reshape(n_cores,
                                                       *out_avals[i].shape)[c]
                 for i, name in enumerate(out_names)}
                for c in range(n_cores)]

    return run


_RUNNER = {}


def _prepare_inputs(x, hidden, emb, Wf, Uf, Wb, Ub, Wp, t_steps):
    """Per-core input dicts. Cores 0-3: forward dir, batch quarters 0-3.
    Cores 4-7: backward dir, batch quarters 0-3."""
    x = np.asarray(x)
    in_maps = []
    for c in range(NCORES):
        bwd = c >= 4
        bq = c % 4
        xs = x[bq * BL:(bq + 1) * BL, :t_steps].astype(np.int64)
        if bwd:
            xs = xs[:, ::-1]
        tokens = xs.T.reshape(-1)                      # i = t*BL + b
        idx = np.tile(tokens.reshape(-1, 16).T.astype(np.int16), (8, 1)).copy()
        mrow = (MASK_BIG * (tokens == 0)).astype(np.float32)[None, :]
        h0T = np.ascontiguousarray(hidden[bq * BL:(bq + 1) * BL].T)
        wp_half = Wp[H:] if bwd else Wp[:H]
        in_maps.append({
            "u": Ub if bwd else Uf,
            "w": Wb if bwd else Wf,
            "emb": emb,
            "idx": idx,
            "mrow": mrow,
            "h0T": h0T.astype(np.float32),
            "wp": np.ascontiguousarray(wp_half).astype(np.float32),
        })
    return in_maps


def _assemble(results, t_steps):
    """outputs: [64, t, 2H] f32 and partial-state sum pieces."""
    nchunk = t_steps // TC
    out = np.empty((B, t_steps, 2 * H), np.float32)
    s_parts = np.zeros((2, 4, H, BL), np.float32)
    for c in range(NCORES):
        bwd = c >= 4
        bq = c % 4
        oT = np.asarray(results[c]["outT"])              # [nchunk, KT, 128, TC, BL]
        o = oT.astype(np.float32).transpose(4, 0, 3, 1, 2)  # [BL, nchunk, TC, KT, 128]
        o = o.reshape(BL, t_steps, H)
        if bwd:
            o = o[:, ::-1, :]
        out[bq * BL:(bq + 1) * BL, :, (H if bwd else 0):(2 * H if bwd else H)] = o
        spc = np.asarray(results[c]["sp"])               # [128, 8*BL]
        s_parts[int(bwd), bq] = spc.reshape(128, 8, BL).transpose(1, 0, 2).reshape(H, BL)
    return out, s_parts


def kernel(x, hidden, emb, Wf, Uf, bf_in, bf_rec, Wb, Ub, bb_in, bb_rec, Wp, bp,
           t_steps=T):
    x = np.asarray(x)
    hidden = np.asarray(hidden, np.float32)
    for b_ in (bf_in, bf_rec, bb_in, bb_rec):
        assert not np.any(np.asarray(b_)), "nonzero biases not enabled"
    nc = _get_program(t_steps, False, False)
    key = (t_steps,)
    if key not in _RUNNER:
        _RUNNER[key] = _make_runner(nc, NCORES)
    in_maps = _prepare_inputs(x, hidden, np.asarray(emb, np.float32),
                              np.asarray(Wf, np.float32), np.asarray(Uf, np.float32),
                              np.asarray(Wb, np.float32), np.asarray(Ub, np.float32),
                              np.asarray(Wp, np.float32), t_steps)
    results = _RUNNER[key](in_maps)
    out, s_parts = _assemble(results, t_steps)
    s = s_parts[0] + s_parts[1]                      # [4, H, BL]
    state = np.tanh(np.concatenate([s[q].T for q in range(4)], axis=0)
                    + np.asarray(bp, np.float32)[None, :])
    return out, state
